# revision 25
# baseline (speedup 1.0000x reference)
"""Trainium2 Bass kernel for nn_AttentionLayer (B=16, TQ=TK=H=1024, fp32).

reference:
    scores  = einsum('bqh,bkh->bqk', query, memory_bank)
    probs   = softmax(scores, axis=2)
    context = einsum('bqk,bkh->bqh', probs, memory_bank)
    return (context, scores)

Sharding: batch dim split across 8 NeuronCores (2 batches per core), no
cross-device communication.

Per-core kernel (per batch):
  - load K natural [k, h] into SBUF; build K^T via PE transposes (rounded to
    fp32r in the PSUM->SBUF copy) and K_r (fp32r copy of K) for the second
    matmul.
  - per 128-row q-tile: transpose Q block to Q^T (fp32r), S = (Q^T)^T @ K^T
    accumulated over h in PSUM, softmax row stats on DVE/ACT (exp produces the
    row sum via accum_out), scores DMA'd straight out of PSUM, E^T via PE
    transposes, C = (E^T)^T @ K_r in PSUM, row-scaled by 1/sum into SBUF, DMA
    out.
All matmuls run in fp32r (TF32-like) at full PE rate; operands are rounded to
fp32r by the copies that stage them into SBUF (required by the BIR verifier).
"""

import numpy as np

import concourse.bass as bass
import concourse.mybir as mybir
import concourse.tile as tile
from concourse import bacc
from concourse.masks import make_identity
from concourse.bass_utils import run_bass_kernel_spmd

N_CORES = 8
B, TQ, TK, H = 16, 1024, 1024, 1024
B_PC = B // N_CORES
P = 128

F32 = mybir.dt.float32
F32R = mybir.dt.float32r


def _mm_chunks(width):
    """Split a free-dim width into <=512 chunks (fp32 moving-operand cap)."""
    n = max(1, (width + 511) // 512)
    assert width % n == 0
    return [(i * (width // n), width // n) for i in range(n)]


def build_attention_nc(b_pc=B_PC, tq=TQ, tk=TK, h=H, repeats=1, strip_dma=False):
    """Build (and compile) the per-core Bass program.

    DRAM tensors: query [b_pc, tq, h], memory_bank [b_pc, tk, h] (inputs);
    scores [b_pc, tq, tk], context [b_pc, tq, h] (outputs). All fp32.
    repeats>1 wraps the whole computation in a hardware loop (timing only).
    """
    nq, nk, nh = tq // P, tk // P, h // P
    assert tq % P == 0 and tk % P == 0 and h % P == 0

    nc = bacc.Bacc("TRN2", debug=False, target_bir_lowering=False)
    q_d = nc.dram_tensor("query", [b_pc, tq, h], F32, kind="ExternalInput").ap()
    k_d = nc.dram_tensor("memory_bank", [b_pc, tk, h], F32, kind="ExternalInput").ap()
    s_d = nc.dram_tensor("scores", [b_pc, tq, tk], F32, kind="ExternalOutput").ap()
    c_d = nc.dram_tensor("context", [b_pc, tq, h], F32, kind="ExternalOutput").ap()

    with tile.TileContext(nc) as tc:
        with (
            tc.tile_pool(name="singles", bufs=1) as singles,
            tc.tile_pool(name="kn", bufs=2) as kn_pool,
            tc.tile_pool(name="kt", bufs=1) as kt_pool,
            tc.tile_pool(name="knr", bufs=1) as knr_pool,
            tc.tile_pool(name="qraw", bufs=2) as qraw_pool,
            tc.tile_pool(name="qt", bufs=2) as qt_pool,
            tc.tile_pool(name="ev", bufs=2) as e_pool,
            tc.tile_pool(name="sout", bufs=2) as s_pool,
            tc.tile_pool(name="et", bufs=2) as et_pool,
            tc.tile_pool(name="cout", bufs=2) as c_pool,
            tc.tile_pool(name="stats", bufs=6) as stats_pool,
            tc.tile_pool(name="ps_s", bufs=2, space="PSUM") as ps_s_pool,
            tc.tile_pool(name="ps_c", bufs=1, space="PSUM") as ps_c_pool,
            tc.tile_pool(name="ps_t", bufs=2, space="PSUM") as ps_t_pool,
        ):
            ident = singles.tile([P, P], F32)
            make_identity(nc, ident)

            def body(_iv=None):
                # one software-pipelined pass over (batch, q-tile)

                def preamble(b):
                    kn = kn_pool.tile([P, nk, h], F32, tag="kn")
                    for j in range(nk):
                        nc.sync.dma_start(
                            out=kn[:, j, :], in_=k_d[b, j * P : (j + 1) * P, :]
                        )
                    # K^T: kt[p, i, j*P:(j+1)*P] = K[j*P+0.., i*P+p]
                    kt = kt_pool.tile([P, nh, tk], F32R, tag="kt")
                    g = 0
                    for j0 in range(0, nk, 4):
                        for i in range(nh):
                            jj = min(4, nk - j0)
                            pt = ps_t_pool.tile([P, 4, P], F32, tag="pt")
                            for j in range(j0, j0 + jj):
                                nc.tensor.transpose(
                                    pt[:, j - j0, :],
                                    kn[:, j, i * P : (i + 1) * P],
                                    ident,
                                )
                            if g % 2 == 0:
                                nc.vector.tensor_copy(
                                    kt[:, i, j0 * P : (j0 + jj) * P], pt[:, :jj, :]
                                )
                            else:
                                nc.scalar.copy(
                                    kt[:, i, j0 * P : (j0 + jj) * P], pt[:, :jj, :]
                                )
                            g += 1
                    # K rounded to fp32r for the context matmul
                    knr = knr_pool.tile([P, nk, h], F32R, tag="knr")
                    for j in range(nk):
                        nc.scalar.copy(knr[:, j, :], kn[:, j, :])
                    return kt, knr, kn

                def produce_qt(b, qt, kn):
                    if strip_dma:
                        qraw = kn[:, qt % nk, :]
                    else:
                        qraw = qraw_pool.tile([P, h], F32, tag="qraw")
                        nc.scalar.dma_start(
                            out=qraw, in_=q_d[b, qt * P : (qt + 1) * P, :]
                        )
                    qtt = qt_pool.tile([P, nh, P], F32R, tag="qt")
                    for g, i0 in enumerate(range(0, nh, 4)):
                        ii = min(4, nh - i0)
                        pt = ps_t_pool.tile([P, 4, P], F32, tag="pt")
                        for i in range(i0, i0 + ii):
                            nc.tensor.transpose(
                                pt[:, i - i0, :], qraw[:, i * P : (i + 1) * P], ident
                            )
                        if g % 2 == 0:
                            nc.scalar.copy(qtt[:, i0 : i0 + ii, :], pt[:, :ii, :])
                        else:
                            nc.vector.tensor_copy(qtt[:, i0 : i0 + ii, :], pt[:, :ii, :])
                    return qtt

                def s_phase(b, qt, qtt, kt):
                    ps_s = ps_s_pool.tile([P, tk], F32, tag="ps_s")
                    negm_parts = []
                    for ci, (off, w) in enumerate(_mm_chunks(tk)):
                        for i in range(nh):
                            nc.tensor.matmul(
                                ps_s[:, off : off + w],
                                qtt[:, i, :],
                                kt[:, i, off : off + w],
                                start=(i == 0),
                                stop=(i == nh - 1),
                            )
                        # per-chunk -max, so the row max is ready right when
                        # the last chunk's matmuls finish
                        nm = stats_pool.tile([P, 1], F32, tag=f"negm{ci}")
                        nc.vector.reduce_max(
                            out=nm,
                            in_=ps_s[:, off : off + w],
                            axis=mybir.AxisListType.X,
                            negate=True,
                        )
                        negm_parts.append(nm)
                    return ps_s, negm_parts

                def softmax_et(b, qt, ps_s, negm_parts):
                    if len(negm_parts) == 1:
                        negm = negm_parts[0]
                    else:
                        negm = stats_pool.tile([P, 1], F32, tag="negm")
                        nc.vector.tensor_tensor(
                            out=negm,
                            in0=negm_parts[0],
                            in1=negm_parts[1],
                            op=mybir.AluOpType.min,
                        )
                        for nm in negm_parts[2:]:
                            nc.vector.tensor_tensor(
                                out=negm, in0=negm, in1=nm, op=mybir.AluOpType.min
                            )
                    ev = e_pool.tile([P, tk], F32, tag="ev")
                    esums = []
                    for ci, (off, w) in enumerate(_mm_chunks(tk)):
                        es = stats_pool.tile([P, 1], F32, tag=f"esum{ci}")
                        nc.vector.memset(es, 0.0)
                        nc.scalar.activation(
                            out=ev[:, off : off + w],
                            in_=ps_s[:, off : off + w],
                            func=mybir.ActivationFunctionType.Exp,
                            bias=negm,
                            scale=1.0,
                            accum_out=es,
                        )
                        esums.append(es)
                    if len(esums) == 1:
                        esum = esums[0]
                    else:
                        esum = stats_pool.tile([P, 1], F32, tag="esum")
                        nc.vector.tensor_add(esum, esums[0], esums[1])
                        for es in esums[2:]:
                            nc.vector.tensor_add(esum, esum, es)
                    # raw scores: PSUM -> SBUF staging -> DRAM
                    sout = s_pool.tile([P, tk], F32, tag="sout")
                    nc.vector.tensor_copy(sout, ps_s)
                    if not strip_dma:
                        nc.gpsimd.dma_start(
                            out=s_d[b, qt * P : (qt + 1) * P, :], in_=sout
                        )
                    r = stats_pool.tile([P, 1], F32, tag="r")
                    nc.vector.reciprocal(r, esum)
                    # E^T tiles
                    ett = et_pool.tile([P, nk, P], F32R, tag="et")
                    for g, j0 in enumerate(range(0, nk, 4)):
                        jj = min(4, nk - j0)
                        pt = ps_t_pool.tile([P, 4, P], F32, tag="pt")
                        for j in range(j0, j0 + jj):
                            nc.tensor.transpose(
                                pt[:, j - j0, :], ev[:, j * P : (j + 1) * P], ident
                            )
                        if g % 2 == 0:
                            nc.vector.tensor_copy(ett[:, j0 : j0 + jj, :], pt[:, :jj, :])
                        else:
                            nc.scalar.copy(ett[:, j0 : j0 + jj, :], pt[:, :jj, :])
                    return ett, r

                def c_phase(b, qt, ett, r, knr):
                    ps_c = ps_c_pool.tile([P, h], F32, tag="ps_c")
                    for off, w in _mm_chunks(h):
                        for j in range(nk):
                            nc.tensor.matmul(
                                ps_c[:, off : off + w],
                                ett[:, j, :],
                                knr[:, j, off : off + w],
                                start=(j == 0),
                                stop=(j == nk - 1),
                            )
                    cout = c_pool.tile([P, h], F32, tag="cout")
                    nc.vector.tensor_scalar_mul(cout, ps_c, r)
                    if not strip_dma or (b == b_pc - 1 and qt == nq - 1):
                        nc.gpsimd.dma_start(
                            out=c_d[b, qt * P : (qt + 1) * P, :], in_=cout
                        )

                # Pipelined emission, grouped so the PE alternates one
                # transpose phase (QTt(t) + ETt(t-1)) with one long matmul
                # phase (C(t-1) + S(t)) to keep the PE clock-gate warm.
                tiles = [(b, qt) for b in range(b_pc) for qt in range(nq)]
                prev = None  # (b, qt, ps_s, knr)
                cur = {}
                for b, qt in tiles:
                    if qt == 0:
                        cur[b] = preamble(b)
                    qtt = produce_qt(b, qt, cur[b][2])
                    if prev is not None:
                        pb, pqt, ps_s_prev, pnm, pknr = prev
                        ett, r = softmax_et(pb, pqt, ps_s_prev, pnm)
                        c_phase(pb, pqt, ett, r, pknr)
                    ps_s, negm_parts = s_phase(b, qt, qtt, cur[b][0])
                    prev = (b, qt, ps_s, negm_parts, cur[b][1])
                pb, pqt, ps_s_prev, pnm, pknr = prev
                ett, r = softmax_et(pb, pqt, ps_s_prev, pnm)
                c_phase(pb, pqt, ett, r, pknr)

            if repeats == 1:
                body()
            else:
                with tc.For_i(
                    0, repeats, 1, hint_engines=(mybir.EngineType.PE,)
                ) as iv:
                    body(iv)

    nc.compile()
    return nc


_NC_CACHE = {}


def _get_nc(repeats=1):
    key = repeats
    if key not in _NC_CACHE:
        _NC_CACHE[key] = build_attention_nc(repeats=repeats)
    return _NC_CACHE[key]


def run_on_hw(query, memory_bank, repeats=1):
    nc = _get_nc(repeats)
    query = np.ascontiguousarray(query, dtype=np.float32)
    memory_bank = np.ascontiguousarray(memory_bank, dtype=np.float32)
    in_maps = [
        {
            "query": query[c * B_PC : (c + 1) * B_PC],
            "memory_bank": memory_bank[c * B_PC : (c + 1) * B_PC],
        }
        for c in range(N_CORES)
    ]
    res = run_bass_kernel_spmd(nc, in_maps, core_ids=list(range(N_CORES)))
    context = np.concatenate([res.results[c]["context"] for c in range(N_CORES)], axis=0)
    scores = np.concatenate([res.results[c]["scores"] for c in range(N_CORES)], axis=0)
    return context, scores


def kernel(query, memory_bank):
    return run_on_hw(query, memory_bank, repeats=1)


# revision 28
# speedup vs baseline: 1.0720x; 1.0720x over previous
"""Trainium2 Bass kernel for nn_AttentionLayer (B=16, TQ=TK=H=1024, fp32).

reference:
    scores  = einsum('bqh,bkh->bqk', query, memory_bank)
    probs   = softmax(scores, axis=2)
    context = einsum('bqk,bkh->bqh', probs, memory_bank)
    return (context, scores)

Sharding: batch dim split across 8 NeuronCores (2 batches per core), no
cross-device communication.

Per-core kernel (per batch):
  - load K natural [k, h] into SBUF; build K^T via PE transposes (rounded to
    fp32r in the PSUM->SBUF copy) and K_r (fp32r copy of K) for the second
    matmul.
  - per 128-row q-tile: transpose Q block to Q^T (fp32r), S = (Q^T)^T @ K^T
    accumulated over h in PSUM, softmax row stats on DVE/ACT (exp produces the
    row sum via accum_out), scores DMA'd straight out of PSUM, E^T via PE
    transposes, C = (E^T)^T @ K_r in PSUM, row-scaled by 1/sum into SBUF, DMA
    out.
All matmuls run in fp32r (TF32-like) at full PE rate; operands are rounded to
fp32r by the copies that stage them into SBUF (required by the BIR verifier).
"""

import numpy as np

import concourse.bass as bass
import concourse.mybir as mybir
import concourse.tile as tile
from concourse import bacc
from concourse.masks import make_identity
from concourse.bass_utils import run_bass_kernel_spmd

N_CORES = 8
B, TQ, TK, H = 16, 1024, 1024, 1024
B_PC = B // N_CORES
P = 128

F32 = mybir.dt.float32
F32R = mybir.dt.float32r


def _mm_chunks(width):
    """Split a free-dim width into <=512 chunks (fp32 moving-operand cap)."""
    n = max(1, (width + 511) // 512)
    assert width % n == 0
    return [(i * (width // n), width // n) for i in range(n)]


def build_attention_nc(b_pc=B_PC, tq=TQ, tk=TK, h=H, repeats=1, strip_dma=False):
    """Build (and compile) the per-core Bass program.

    DRAM tensors: query [b_pc, tq, h], memory_bank [b_pc, tk, h] (inputs);
    scores [b_pc, tq, tk], context [b_pc, tq, h] (outputs). All fp32.
    repeats>1 wraps the whole computation in a hardware loop (timing only).
    """
    nq, nk, nh = tq // P, tk // P, h // P
    assert tq % P == 0 and tk % P == 0 and h % P == 0

    nc = bacc.Bacc("TRN2", debug=False, target_bir_lowering=False)
    q_d = nc.dram_tensor("query", [b_pc, tq, h], F32, kind="ExternalInput").ap()
    k_d = nc.dram_tensor("memory_bank", [b_pc, tk, h], F32, kind="ExternalInput").ap()
    s_d = nc.dram_tensor("scores", [b_pc, tq, tk], F32, kind="ExternalOutput").ap()
    c_d = nc.dram_tensor("context", [b_pc, tq, h], F32, kind="ExternalOutput").ap()

    with tile.TileContext(nc) as tc:
        with (
            tc.tile_pool(name="singles", bufs=1) as singles,
            tc.tile_pool(name="kn", bufs=1) as kn_pool,
            tc.tile_pool(name="kt", bufs=2) as kt_pool,
            tc.tile_pool(name="knr", bufs=1) as knr_pool,
            tc.tile_pool(name="qraw", bufs=2) as qraw_pool,
            tc.tile_pool(name="qt", bufs=2) as qt_pool,
            tc.tile_pool(name="ev", bufs=2) as e_pool,
            tc.tile_pool(name="sout", bufs=2) as s_pool,
            tc.tile_pool(name="et", bufs=2) as et_pool,
            tc.tile_pool(name="cout", bufs=2) as c_pool,
            tc.tile_pool(name="stats", bufs=6) as stats_pool,
            tc.tile_pool(name="ps_s", bufs=2, space="PSUM") as ps_s_pool,
            tc.tile_pool(name="ps_c", bufs=1, space="PSUM") as ps_c_pool,
            tc.tile_pool(name="ps_t", bufs=2, space="PSUM") as ps_t_pool,
        ):
            ident = singles.tile([P, P], F32)
            make_identity(nc, ident)

            def body(_iv=None):
                # one software-pipelined pass over (batch, q-tile)

                def pre_kn(b):
                    kn = kn_pool.tile([P, nk, h], F32, tag="kn")
                    for j in range(nk):
                        nc.sync.dma_start(
                            out=kn[:, j, :], in_=k_d[b, j * P : (j + 1) * P, :]
                        )
                    return kn

                def pre_kt_alloc():
                    # K^T: kt[p, i, j*P:(j+1)*P] = K[j*P+0.., i*P+p]
                    kt = kt_pool.tile([P, nh, tk], F32R, tag="kt")
                    return kt

                def pre_kt_half(kn, kt, j0):
                    jj = min(4, nk - j0)
                    for i in range(nh):
                        pt = ps_t_pool.tile([P, 4, P], F32, tag="pt")
                        for j in range(j0, j0 + jj):
                            nc.tensor.transpose(
                                pt[:, j - j0, :],
                                kn[:, j, i * P : (i + 1) * P],
                                ident,
                            )
                        if i % 2 == 0:
                            nc.vector.tensor_copy(
                                kt[:, i, j0 * P : (j0 + jj) * P], pt[:, :jj, :]
                            )
                        else:
                            nc.scalar.copy(
                                kt[:, i, j0 * P : (j0 + jj) * P], pt[:, :jj, :]
                            )

                def pre_knr(kn):
                    # K rounded to fp32r for the context matmul
                    knr = knr_pool.tile([P, nk, h], F32R, tag="knr")
                    for j in range(nk):
                        nc.scalar.copy(knr[:, j, :], kn[:, j, :])
                    return knr

                def preamble(b):
                    kn = pre_kn(b)
                    kt = pre_kt_alloc()
                    for j0 in range(0, nk, 4):
                        pre_kt_half(kn, kt, j0)
                    knr = pre_knr(kn)
                    return kt, knr, kn

                def produce_qt(b, qt, kn):
                    if strip_dma:
                        qraw = kn[:, qt % nk, :]
                    else:
                        qraw = qraw_pool.tile([P, h], F32, tag="qraw")
                        nc.sync.dma_start(
                            out=qraw, in_=q_d[b, qt * P : (qt + 1) * P, :]
                        )
                    qtt = qt_pool.tile([P, nh, P], F32R, tag="qt")
                    for g, i0 in enumerate(range(0, nh, 4)):
                        ii = min(4, nh - i0)
                        pt = ps_t_pool.tile([P, 4, P], F32, tag="pt")
                        for i in range(i0, i0 + ii):
                            nc.tensor.transpose(
                                pt[:, i - i0, :], qraw[:, i * P : (i + 1) * P], ident
                            )
                        if g % 2 == 0:
                            nc.scalar.copy(qtt[:, i0 : i0 + ii, :], pt[:, :ii, :])
                        else:
                            nc.vector.tensor_copy(qtt[:, i0 : i0 + ii, :], pt[:, :ii, :])
                    return qtt

                def s_phase(b, qt, qtt, kt):
                    ps_s = ps_s_pool.tile([P, tk], F32, tag="ps_s")
                    negm_parts = []
                    for ci, (off, w) in enumerate(_mm_chunks(tk)):
                        for i in range(nh):
                            nc.tensor.matmul(
                                ps_s[:, off : off + w],
                                qtt[:, i, :],
                                kt[:, i, off : off + w],
                                start=(i == 0),
                                stop=(i == nh - 1),
                            )
                        # per-chunk -max, so the row max is ready right when
                        # the last chunk's matmuls finish
                        nm = stats_pool.tile([P, 1], F32, tag=f"negm{ci}")
                        nc.vector.reduce_max(
                            out=nm,
                            in_=ps_s[:, off : off + w],
                            axis=mybir.AxisListType.X,
                            negate=True,
                        )
                        negm_parts.append(nm)
                    return ps_s, negm_parts

                def softmax_et(b, qt, ps_s, negm_parts):
                    if len(negm_parts) == 1:
                        negm = negm_parts[0]
                    else:
                        negm = stats_pool.tile([P, 1], F32, tag="negm")
                        nc.vector.tensor_tensor(
                            out=negm,
                            in0=negm_parts[0],
                            in1=negm_parts[1],
                            op=mybir.AluOpType.min,
                        )
                        for nm in negm_parts[2:]:
                            nc.vector.tensor_tensor(
                                out=negm, in0=negm, in1=nm, op=mybir.AluOpType.min
                            )
                    ev = e_pool.tile([P, tk], F32, tag="ev")
                    esums = []
                    for ci, (off, w) in enumerate(_mm_chunks(tk)):
                        es = stats_pool.tile([P, 1], F32, tag=f"esum{ci}")
                        nc.vector.memset(es, 0.0)
                        nc.scalar.activation(
                            out=ev[:, off : off + w],
                            in_=ps_s[:, off : off + w],
                            func=mybir.ActivationFunctionType.Exp,
                            bias=negm,
                            scale=1.0,
                            accum_out=es,
                        )
                        esums.append(es)
                    if len(esums) == 1:
                        esum = esums[0]
                    else:
                        esum = stats_pool.tile([P, 1], F32, tag="esum")
                        nc.vector.tensor_add(esum, esums[0], esums[1])
                        for es in esums[2:]:
                            nc.vector.tensor_add(esum, esum, es)
                    # raw scores: PSUM -> SBUF staging -> DRAM
                    sout = s_pool.tile([P, tk], F32, tag="sout")
                    nc.vector.tensor_copy(sout, ps_s)
                    if not strip_dma:
                        nc.gpsimd.dma_start(
                            out=s_d[b, qt * P : (qt + 1) * P, :], in_=sout
                        )
                    r = stats_pool.tile([P, 1], F32, tag="r")
                    nc.vector.reciprocal(r, esum)
                    # E^T tiles
                    ett = et_pool.tile([P, nk, P], F32R, tag="et")
                    for g, j0 in enumerate(range(0, nk, 4)):
                        jj = min(4, nk - j0)
                        pt = ps_t_pool.tile([P, 4, P], F32, tag="pt")
                        for j in range(j0, j0 + jj):
                            nc.tensor.transpose(
                                pt[:, j - j0, :], ev[:, j * P : (j + 1) * P], ident
                            )
                        if g % 2 == 0:
                            nc.vector.tensor_copy(ett[:, j0 : j0 + jj, :], pt[:, :jj, :])
                        else:
                            nc.scalar.copy(ett[:, j0 : j0 + jj, :], pt[:, :jj, :])
                    return ett, r

                def c_phase(b, qt, ett, r, knr):
                    ps_c = ps_c_pool.tile([P, h], F32, tag="ps_c")
                    for off, w in _mm_chunks(h):
                        for j in range(nk):
                            nc.tensor.matmul(
                                ps_c[:, off : off + w],
                                ett[:, j, :],
                                knr[:, j, off : off + w],
                                start=(j == 0),
                                stop=(j == nk - 1),
                            )
                    cout = c_pool.tile([P, h], F32, tag="cout")
                    nc.vector.tensor_scalar_mul(cout, ps_c, r)
                    if not strip_dma or (b == b_pc - 1 and qt == nq - 1):
                        nc.gpsimd.dma_start(
                            out=c_d[b, qt * P : (qt + 1) * P, :], in_=cout
                        )

                # Pipelined emission, grouped so the PE alternates one
                # transpose phase (QTt(t) + ETt(t-1)) with one long matmul
                # phase (C(t-1) + S(t)) to keep the PE clock-gate warm.
                tiles = [(b, qt) for b in range(b_pc) for qt in range(nq)]
                prev = None  # (b, qt, ps_s, knr)
                cur = {}
                pending = {}  # partially-built next-batch state
                for b, qt in tiles:
                    if b == 0 and qt == 0:
                        cur[0] = preamble(0)
                    elif qt == 0:
                        # finish any slices not emitted during the previous batch
                        pk = pending.pop(b, None)
                        if pk is None:
                            cur[b] = preamble(b)
                        else:
                            cur[b] = (pk["kt"], pre_knr(pk["kn"]), pk["kn"])
                    nxt = b + 1
                    if nxt < b_pc and nq >= 8:
                        # spread next batch's K load + K^T build over this batch
                        if qt == nq - 4:
                            pending[nxt] = {"kn": pre_kn(nxt)}
                        elif qt == nq - 3:
                            pending[nxt]["kt"] = pre_kt_alloc()
                            pre_kt_half(pending[nxt]["kn"], pending[nxt]["kt"], 0)
                        elif qt == nq - 2:
                            pre_kt_half(pending[nxt]["kn"], pending[nxt]["kt"], 4)
                    qtt = produce_qt(b, qt, cur[b][2])
                    if prev is not None:
                        pb, pqt, ps_s_prev, pnm, pknr = prev
                        ett, r = softmax_et(pb, pqt, ps_s_prev, pnm)
                        c_phase(pb, pqt, ett, r, pknr)
                    ps_s, negm_parts = s_phase(b, qt, qtt, cur[b][0])
                    prev = (b, qt, ps_s, negm_parts, cur[b][1])
                pb, pqt, ps_s_prev, pnm, pknr = prev
                ett, r = softmax_et(pb, pqt, ps_s_prev, pnm)
                c_phase(pb, pqt, ett, r, pknr)

            if repeats == 1:
                body()
            else:
                with tc.For_i(
                    0, repeats, 1, hint_engines=(mybir.EngineType.PE,)
                ) as iv:
                    body(iv)

    nc.compile()
    return nc


_NC_CACHE = {}


def _get_nc(repeats=1):
    key = repeats
    if key not in _NC_CACHE:
        _NC_CACHE[key] = build_attention_nc(repeats=repeats)
    return _NC_CACHE[key]


def run_on_hw(query, memory_bank, repeats=1):
    nc = _get_nc(repeats)
    query = np.ascontiguousarray(query, dtype=np.float32)
    memory_bank = np.ascontiguousarray(memory_bank, dtype=np.float32)
    in_maps = [
        {
            "query": query[c * B_PC : (c + 1) * B_PC],
            "memory_bank": memory_bank[c * B_PC : (c + 1) * B_PC],
        }
        for c in range(N_CORES)
    ]
    res = run_bass_kernel_spmd(nc, in_maps, core_ids=list(range(N_CORES)))
    context = np.concatenate([res.results[c]["context"] for c in range(N_CORES)], axis=0)
    scores = np.concatenate([res.results[c]["scores"] for c in range(N_CORES)], axis=0)
    return context, scores


def kernel(query, memory_bank):
    return run_on_hw(query, memory_bank, repeats=1)
